# revision 24
# baseline (speedup 1.0000x reference)
"""Causal self-attention TP kernel for 8 trn2 NeuronCores.

Problem shapes (hardcoded): x [2, 2048, 2048] f32, w_attn [2048, 6144],
w_proj [2048, 2048], 16 heads, head_dim 128.

Sharding: tensor-parallel over heads - core i owns heads {2i, 2i+1} for BOTH
batches. Each core computes its local-head qkv + attention, producing
y_local^T [128 feat, 2048 tok] per (batch, head). One 8-core AllToAll per
(batch, head) re-shards from feature-split to token-split: core g receives
y^T[all 2048 feat, 256 tokens of each batch] and projects those 512 tokens
against the full w_proj, emitting out[512, 2048] (batch0 rows then batch1).

Software-pipelined schedule (per core): causal attention for q-tile j
only needs k/v token tiles <= j, so attention iterations for q-tile j
are WOVEN between the qkv matmul groups of token tile j+1. The PE-bound
qkv phases absorb the ACT (exp) latency of the attention chain, so
neither engine ever drains. The per-(batch,head) AllToAlls fire as each
head's last q-tile completes: batch-0's hide under batch-1's qkv,
batch-1's under proj(b0). Both batches' V stays in SBUF. Everything is
bf16 (converted host-side) except PSUM accumulation (f32).

Queue discipline: SP (sync) queue carries x/wqk/wv loads and the y_t
loads (issued only once the SP queue is otherwise drained, and always
AFTER the a2a that writes them - the tile framework only creates the
RAW dep for writers already emitted); w_proj chunks and out stores ride
the ACT queue (pure loads/stores that never park it at a bad time);
y stores + collectives + constants ride the Pool (SWDGE) queue. A
parked collective never blocks a latency-critical load.

Attention: no max-subtraction softmax (scores ~N(0,1)); row sums via a
per-tile ones-matmul on the PE; reciprocal broadcast applied to y after
the PV matmul. The inner loop is software-pipelined one step (QK_{c+1}
issued before PV_c/R_c). Diagonal 128x512 score tiles restrict
QK/exp/PV/rowsum to the live [128*m:512] column range.

reps>1 repeats the whole kernel body with an all-engine barrier between
reps - used for timing the device portion free of the ~1ms/call axon
tunnel dispatch cost.
"""

from collections import deque

import numpy as np
import ml_dtypes

import concourse.bass as bass
import concourse.mybir as mybir
import concourse.tile as tile
from concourse import bacc
from concourse.bass_utils import run_bass_kernel_spmd

F32 = mybir.dt.float32
BF16 = mybir.dt.bfloat16
NPBF16 = ml_dtypes.bfloat16

B, T, C = 2, 2048, 2048
H, D = 16, 128
NTOK = B * T                     # 4096 flat tokens (batch-major)
SCALE = 1.0 / float(np.sqrt(D))
NCORES = 8
HPC = H // NCORES                # 2 heads per core
FLOC = HPC * D                   # 256 local v features
QK = 512                         # q+k local features (2 heads x 128 x 2)

last_exec_time_ns = None
_cache = {}


def _tri_np():
    # tri[kk, qq] = 1.0 iff kk <= qq  (lower-triangular causal mask, 128x128)
    kk = np.arange(128)[:, None]
    qq = np.arange(128)[None, :]
    return (kk <= qq).astype(NPBF16)


def build_nc(no_collective=False, reps=1):
    nc = bacc.Bacc("TRN2", target_bir_lowering=False, debug=False,
                   num_devices=1 if no_collective else NCORES)

    xt = nc.dram_tensor("xt", [C, NTOK], BF16, kind="ExternalInput")
    wqk = nc.dram_tensor("wqk", [C, QK], BF16, kind="ExternalInput")
    wv = nc.dram_tensor("wv", [C, FLOC], BF16, kind="ExternalInput")
    wp = nc.dram_tensor("wp", [C, C], BF16, kind="ExternalInput")
    out = nc.dram_tensor("out", [512, C], F32, kind="ExternalOutput")

    # per-(batch, head) a2a buffers: 8 shards x [128 feat x 256 tok]
    y_loc = {(b, h): nc.dram_tensor(f"y_loc{b}{h}", [1024, 256], BF16)
             for b in range(B) for h in range(HPC)}
    y_t = {(b, h): nc.dram_tensor(f"y_t{b}{h}", [1024, 256], BF16)
           for b in range(B) for h in range(HPC)}
    tri_dr = nc.inline_tensor(_tri_np(), "tri_c")
    ones_dr = nc.inline_tensor(np.ones((128, 1), NPBF16), "ones_c")
    zeros_dr = nc.inline_tensor(np.zeros((128, 1), np.float32), "zeros_c")

    def a2a(b, h):
        if no_collective:
            nc.gpsimd.dma_start(out=y_t[(b, h)][:, :], in_=y_loc[(b, h)][:, :])
        else:
            nc.gpsimd.collective_compute(
                "AllToAll",
                mybir.AluOpType.bypass,
                replica_groups=[list(range(NCORES))],
                ins=[y_loc[(b, h)][:, :]],
                outs=[y_t[(b, h)][:, :]],
            )

    with tile.TileContext(nc) as tc:
        with tc.tile_pool(name="persist", bufs=1) as persist:
            # q^T,k^T for 2 heads, all tokens: chunk f = {q_h0, q_h1, k_h0, k_h1}
            qk_res = persist.tile([128, 4, NTOK], BF16)
            # v, token-major, per (batch, head): [128 tok, 16 chunks, 128 feat]
            v_pre = {(b, h): persist.tile([128, 16, 128], BF16,
                                          name=f"v_pre{b}{h}")
                     for b in range(B) for h in range(HPC)}
            ones_sb = persist.tile([128, 1], BF16)
            zeros_sb = persist.tile([128, 1], F32)
            tri_sb = persist.tile([128, 128], BF16)
            scr = persist.tile([128, 1], F32)
            # constants ride the Pool (SWDGE) queue; SP queue is reserved for
            # the latency-critical weight/x loads at startup
            nc.gpsimd.dma_start(out=zeros_sb, in_=zeros_dr.ap())
            nc.gpsimd.dma_start(out=ones_sb, in_=ones_dr.ap())
            nc.gpsimd.dma_start(out=tri_sb, in_=tri_dr.ap())
            # warm the ACT exp table set (~2.7us) before attention needs it
            nc.scalar.activation(scr, zeros_sb,
                                 mybir.ActivationFunctionType.Exp,
                                 bias=zeros_sb)

            # long-lived pools open first; phase-1 pools open innermost so
            # they can be released (LIFO) mid-rep to free PSUM banks
            p2p_ctx = tc.tile_pool(name="p2p", bufs=5)
            p2y_ctx = tc.tile_pool(name="p2y", bufs=2)
            p2r_ctx = tc.tile_pool(name="p2r", bufs=2)
            p2pss_ctx = tc.tile_pool(name="p2pss", bufs=3, space="PSUM")
            p2psy_ctx = tc.tile_pool(name="p2psy", bufs=2, space="PSUM")
            p2psr_ctx = tc.tile_pool(name="p2psr", bufs=1, space="PSUM")
            p2p = p2p_ctx.__enter__()
            p2y = p2y_ctx.__enter__()
            p2r = p2r_ctx.__enter__()
            p2pss = p2pss_ctx.__enter__()
            p2psy = p2psy_ctx.__enter__()
            p2psr = p2psr_ctx.__enter__()
            p4w_ctx = tc.tile_pool(name="p4w", bufs=8)
            p4y_ctx = tc.tile_pool(name="p4y", bufs=1)
            p4s_ctx = tc.tile_pool(name="p4s", bufs=2)
            p4e_ctx = tc.tile_pool(name="p4e", bufs=16)
            p4w = p4w_ctx.__enter__()
            p4y = p4y_ctx.__enter__()
            p4s = p4s_ctx.__enter__()
            p4e = p4e_ctx.__enter__()

            def attn_steps(b, j):
                """Generator: attention for q-tile j of batch b, both heads.
                Each next() emits one c-iteration (QK+exp+mask+prev-flush);
                the final step per head emits flush/normalize/stores (+a2a
                after the head's last q-tile)."""
                tok0 = b * T
                nk = 4 * j + 4
                for h in range(HPC):
                    v_sb = v_pre[(b, h)]
                    qf, kf = h, 2 + h
                    y_ps = p2psy.tile([128, 512], F32, tag="yps", name="y_ps")
                    r_ps = p2psr.tile([1, 512], F32, tag="rps", name="r_ps")
                    qs = qk_res[:, qf, tok0 + j * 512: tok0 + (j + 1) * 512]
                    pend = None

                    def flush(stop):
                        pc, pp, plo = pend
                        nc.tensor.matmul(
                            y_ps[:, plo:], lhsT=v_sb[:, pc, :],
                            rhs=pp[:, plo:],
                            start=(pc == 0), stop=stop)
                        nc.tensor.matmul(
                            r_ps[:, plo:], lhsT=ones_sb,
                            rhs=pp[:, plo:],
                            start=(pc == 0), stop=stop)

                    for c in range(nk):
                        m = c - 4 * j          # >= 0 on diagonal tiles
                        lo = 128 * m if m >= 0 else 0
                        s_ps = p2pss.tile([128, 512], F32, tag="sps",
                                          name="s_ps")
                        nc.tensor.matmul(
                            s_ps[:, lo:],
                            lhsT=qk_res[:, kf,
                                        tok0 + c * 128: tok0 + (c + 1) * 128],
                            rhs=qs[:, lo:],
                            start=True, stop=True,
                        )
                        p_sb = p2p.tile([128, 512], BF16, tag="p", name="p_sb")
                        nc.scalar.activation(
                            p_sb[:, lo:], s_ps[:, lo:],
                            mybir.ActivationFunctionType.Exp,
                            scale=SCALE, bias=zeros_sb,
                        )
                        if m >= 0:
                            # only the leading 128 cols are partial
                            nc.vector.tensor_mul(
                                p_sb[:, lo:lo + 128],
                                p_sb[:, lo:lo + 128], tri_sb)
                        if pend is not None:
                            flush(stop=False)
                        pend = (c, p_sb, lo)
                        yield
                    flush(stop=True)
                    rr = p2r.tile([1, 512], F32, tag="rr", name="rr")
                    nc.vector.reciprocal(rr, r_ps)
                    rb = p2r.tile([128, 512], F32, tag="rb", name="rb")
                    nc.gpsimd.partition_broadcast(rb, rr)
                    y_sb = p2y.tile([128, 512], BF16, tag="ysb", name="y_sb")
                    nc.vector.tensor_mul(y_sb, y_ps, rb)
                    # token eighths 2j, 2j+1 of batch b -> y_loc rows, one
                    # DMA via the d-major view (rows = s*128 + d)
                    nc.gpsimd.dma_start(
                        out=y_loc[(b, h)].ap()
                        .rearrange("(s d) t -> d s t", d=128)[:, 2 * j:2 * j + 2, :],
                        in_=y_sb.rearrange("d (e t) -> d e t", e=2),
                    )
                    if j == 3:
                        a2a(b, h)
                    yield

            # ---- weave machinery: drain attention steps between groups ----
            work = deque()          # generators with steps left

            def drain(n):
                done = 0
                while work and done < n:
                    try:
                        next(work[0])
                        done += 1
                    except StopIteration:
                        work.popleft()
                return done

            def drain_all():
                while work:
                    drain(1 << 20)

            def load_yt(b, eng):
                # MUST be issued after the matching a2a()s (the RAW dep on
                # y_t only exists once the collective is in the program);
                # pick a queue where parking on that dep blocks nothing.
                ybs = []
                for h in range(HPC):
                    yb = p4y.tile([128, 8, 256], BF16, tag=f"yt{b}{h}",
                                  name=f"yt{b}{h}")
                    eng.dma_start(
                        out=yb,
                        in_=y_t[(b, h)].ap().rearrange("(n p) t -> p n t",
                                                       p=128))
                    ybs.append(yb)
                return ybs

            def proj_group(b, ybs, wt, ch, tb, ps_pool):
                ps = ps_pool.tile([128, 256], F32, tag="ops", name="ops")
                for c in range(16):
                    nc.tensor.matmul(
                        ps,
                        lhsT=ybs[c % 2][:, c // 2, tb * 128:(tb + 1) * 128],
                        rhs=wt[:, c, :],
                        start=(c == 0), stop=(c == 15),
                    )
                st = p4s.tile([128, 256], F32, tag="ost", name="ost")
                nc.vector.tensor_copy(st, ps)
                nc.scalar.dma_start(
                    out=out[b * 256 + tb * 128: b * 256 + (tb + 1) * 128,
                            ch * 256:(ch + 1) * 256],
                    in_=st,
                )

            for rep in range(reps):
                if rep:
                    tc.strict_bb_all_engine_barrier()

                p1w_ctx = tc.tile_pool(name="p1w", bufs=1)
                p1x_ctx = tc.tile_pool(name="p1x", bufs=3)
                p1ps_ctx = tc.tile_pool(name="p1ps", bufs=2, space="PSUM")
                p1w = p1w_ctx.__enter__()
                p1x = p1x_ctx.__enter__()
                p1ps = p1ps_ctx.__enter__()
                wqk_sb = p1w.tile([128, 16, QK], BF16)
                wv_sb = p1w.tile([128, 16, FLOC], BF16)

                def p1_tile(bi, ti):
                    """qkv for token tile tt (512 tokens), attention steps of
                    the previous q-tile woven between the 8 matmul groups."""
                    tt = 4 * bi + ti
                    xh = [p1x.tile([128, 8, 512], BF16, tag="xh", name="xbuf")
                          for half in range(2)]
                    if tt == 0 and rep == 0:
                        # first tile: interleave weight/x quarters so the
                        # first matmul group starts ~4us earlier
                        for piece in range(4):
                            nc.sync.dma_start(
                                out=wqk_sb[:, piece * 4:(piece + 1) * 4, :],
                                in_=wqk[piece * 512:(piece + 1) * 512, :]
                                .rearrange("(n p) f -> p n f", p=128))
                            half, sub = piece // 2, piece % 2
                            c0 = half * 8 + sub * 4
                            nc.sync.dma_start(
                                out=xh[half][:, sub * 4:(sub + 1) * 4, :],
                                in_=xt[c0 * 128:(c0 + 4) * 128, 0:512]
                                .rearrange("(n p) f -> p n f", p=128))
                        nc.sync.dma_start(
                            out=wv_sb,
                            in_=wv.ap().rearrange("(n p) f -> p n f", p=128))
                    else:
                        if tt == 0:
                            nc.sync.dma_start(
                                out=wqk_sb,
                                in_=wqk.ap().rearrange("(n p) f -> p n f",
                                                       p=128))
                            nc.sync.dma_start(
                                out=wv_sb,
                                in_=wv.ap().rearrange("(n p) f -> p n f",
                                                      p=128))
                        for half in range(2):
                            nc.sync.dma_start(
                                out=xh[half],
                                in_=xt[half * 1024:(half + 1) * 1024,
                                       tt * 512:(tt + 1) * 512]
                                .rearrange("(n p) f -> p n f", p=128))

                    for fb in range(4):
                        ps = p1ps.tile([128, 512], F32, tag="ps", name="ps")
                        for c in range(16):
                            nc.tensor.matmul(
                                ps,
                                lhsT=wqk_sb[:, c, fb * 128:(fb + 1) * 128],
                                rhs=xh[c // 8][:, c % 8, :],
                                start=(c == 0), stop=(c == 15),
                            )
                        nc.vector.tensor_copy(
                            qk_res[:, fb, tt * 512:(tt + 1) * 512], ps)
                        steps_left[0] -= drain(
                            (steps_left[0] + (7 - fb)) // (8 - fb))
                    for tb in range(4):
                        psv = p1ps.tile([128, 512], F32, tag="ps", name="ps")
                        for c in range(16):
                            nc.tensor.matmul(
                                psv[:, :FLOC],
                                lhsT=xh[c // 8][:, c % 8,
                                                tb * 128:(tb + 1) * 128],
                                rhs=wv_sb[:, c, :],
                                start=(c == 0), stop=(c == 15),
                            )
                        for h in range(HPC):
                            nc.vector.tensor_copy(
                                v_pre[(bi, h)][:, ti * 4 + tb, :],
                                psv[:, h * 128:(h + 1) * 128])
                        g = 4 + tb
                        if g == 7:
                            steps_left[0] -= drain(1 << 20)
                        else:
                            steps_left[0] -= drain(
                                (steps_left[0] + (7 - g)) // (8 - g))

                # ================= schedule =================
                steps_left = [0]
                yt0 = yt1 = None
                for bi in range(B):
                    for ti in range(4):
                        p1_tile(bi, ti)
                        work.append(attn_steps(bi, ti))
                        steps_left[0] += 2 * (4 * ti + 4 + 1)
                        if bi == 1:
                            # w_proj chunks stream on the idle ACT queue
                            # during batch-1 qkv (3+3+2 across t5..t7)
                            if ti >= 1:
                                n0 = [0, 0, 3, 6][ti]
                                n1 = [0, 3, 6, 8][ti]
                                for ch in range(n0, n1):
                                    wt = p4w.tile([128, 16, 256], BF16,
                                                  tag="wp", name=f"wp_t{ch}")
                                    nc.scalar.dma_start(
                                        out=wt,
                                        in_=wp[:, ch * 256:(ch + 1) * 256]
                                        .rearrange("(n p) f -> p n f", p=128))
                                    wp_tiles.append(wt)
                    if bi == 0:
                        wp_tiles = []

                # phase-1 pools done (LIFO); free their PSUM banks. The
                # remaining woven steps (A1 j=3) only touch p2 pools.
                p1ps_ctx.__exit__(None, None, None)
                p1x_ctx.__exit__(None, None, None)
                p1w_ctx.__exit__(None, None, None)
                p4ps_ctx = tc.tile_pool(name="p4ps", bufs=2, space="PSUM")
                p4ps = p4ps_ctx.__enter__()

                # A1 j=3 front-loaded; its tail-drain bubbles filled by the
                # first proj(b0) groups; the a2a(1,*) flights are covered by
                # the rest of proj(b0) plus proj(b1)'s even-c half (which
                # only needs yt1[h0], arriving one a2a earlier)
                # batch-0's a2as (emitted in t4's weave) completed long ago;
                # the SP queue is empty, so these load immediately
                yt0 = load_yt(0, nc.sync)
                while steps_left[0] > 4:
                    steps_left[0] -= drain(1)
                pg = [(ch, tb) for ch in range(8) for tb in range(2)]
                for i, (ch, tb) in enumerate(pg[:4]):
                    proj_group(0, yt0, wp_tiles[ch], ch, tb, p4ps)
                    steps_left[0] -= drain(1)
                drain_all()
                steps_left[0] = 0
                yt1 = load_yt(1, nc.sync)
                for ch, tb in pg[4:]:
                    proj_group(0, yt0, wp_tiles[ch], ch, tb, p4ps)
                # proj(b1) pass 1: even c chunks (head-0 features)
                partials = []
                for ch, tb in pg:
                    ps = p4ps.tile([128, 256], F32, tag="ops", name="ops")
                    for i, c in enumerate(range(0, 16, 2)):
                        nc.tensor.matmul(
                            ps,
                            lhsT=yt1[0][:, c // 2, tb * 128:(tb + 1) * 128],
                            rhs=wp_tiles[ch][:, c, :],
                            start=(i == 0), stop=(i == 7),
                        )
                    pe = p4e.tile([128, 256], BF16, tag="pe", name="pe")
                    nc.vector.tensor_copy(pe, ps)
                    partials.append(pe)
                # pass 2: odd c chunks (head-1 features) + combine
                for (ch, tb), pe in zip(pg, partials):
                    ps = p4ps.tile([128, 256], F32, tag="ops", name="ops")
                    for i, c in enumerate(range(1, 16, 2)):
                        nc.tensor.matmul(
                            ps,
                            lhsT=yt1[1][:, c // 2, tb * 128:(tb + 1) * 128],
                            rhs=wp_tiles[ch][:, c, :],
                            start=(i == 0), stop=(i == 7),
                        )
                    st = p4s.tile([128, 256], F32, tag="ost", name="ost")
                    nc.vector.tensor_add(st, ps, pe)
                    nc.scalar.dma_start(
                        out=out[256 + tb * 128: 256 + (tb + 1) * 128,
                                ch * 256:(ch + 1) * 256],
                        in_=st,
                    )
                p4ps_ctx.__exit__(None, None, None)

            for ctx in (p4e_ctx, p4s_ctx, p4y_ctx, p4w_ctx, p2psr_ctx,
                        p2psy_ctx, p2pss_ctx, p2r_ctx, p2y_ctx, p2p_ctx):
                ctx.__exit__(None, None, None)

    nc.compile()
    return nc


def _in_maps(x, w_attn, w_proj):
    xt = np.ascontiguousarray(
        x.reshape(NTOK, C).T).astype(NPBF16)              # [C, NTOK]
    wp = np.ascontiguousarray(w_proj).astype(NPBF16)
    maps = []
    for i in range(NCORES):
        qcols = w_attn[:, FLOC * i: FLOC * (i + 1)]
        kcols = w_attn[:, C + FLOC * i: C + FLOC * (i + 1)]
        vcols = w_attn[:, 2 * C + FLOC * i: 2 * C + FLOC * (i + 1)]
        maps.append({
            "xt": xt,
            "wqk": np.ascontiguousarray(
                np.concatenate([qcols, kcols], axis=1)).astype(NPBF16),
            "wv": np.ascontiguousarray(vcols).astype(NPBF16),
            "wp": wp,
        })
    return maps


def kernel(x, w_attn, w_proj):
    global last_exec_time_ns
    x = np.asarray(x, dtype=np.float32)
    w_attn = np.asarray(w_attn, dtype=np.float32)
    w_proj = np.asarray(w_proj, dtype=np.float32)

    if "nc" not in _cache:
        _cache["nc"] = build_nc()
    nc = _cache["nc"]

    res = run_bass_kernel_spmd(nc, _in_maps(x, w_attn, w_proj),
                               list(range(NCORES)))
    last_exec_time_ns = res.exec_time_ns

    return assemble([res.results[g]["out"] for g in range(NCORES)])


def assemble(outs):
    # core g's out rows: [0:256] = batch0 tokens [256g:256(g+1)],
    #                    [256:512] = batch1 tokens [256g:256(g+1)]
    full = np.empty((B, T, C), np.float32)
    for g in range(NCORES):
        for b in range(B):
            full[b, 256 * g: 256 * (g + 1), :] = outs[g][b * 256:(b + 1) * 256]
    return full


# revision 29
# speedup vs baseline: 1.0350x; 1.0350x over previous
"""Causal self-attention TP kernel for 8 trn2 NeuronCores.

Problem shapes (hardcoded): x [2, 2048, 2048] f32, w_attn [2048, 6144],
w_proj [2048, 2048], 16 heads, head_dim 128.

Sharding: tensor-parallel over heads - core i owns heads {2i, 2i+1} for BOTH
batches. Each core computes its local-head qkv + attention, producing
y_local^T [128 feat, 2048 tok] per (batch, head). One 8-core AllToAll per
(batch, head) re-shards from feature-split to token-split: core g receives
y^T[all 2048 feat, 256 tokens of each batch] and projects those 512 tokens
against the full w_proj, emitting out[512, 2048] (batch0 rows then batch1).

Software-pipelined schedule (per core): causal attention for q-tile j
only needs k/v token tiles <= j, so attention iterations for q-tile j
are WOVEN between the qkv matmul groups of token tile j+1. The PE-bound
qkv phases absorb the ACT (exp) latency of the attention chain, so
neither engine ever drains. The per-(batch,head) AllToAlls fire as each
head's last q-tile completes: batch-0's hide under batch-1's qkv,
batch-1's under proj(b0). Both batches' V stays in SBUF. Everything is
bf16 (converted host-side) except PSUM accumulation (f32).

Queue discipline: SP (sync) queue carries x/wqk/wv loads and the y_t
loads (issued only once the SP queue is otherwise drained, and always
AFTER the a2a that writes them - the tile framework only creates the
RAW dep for writers already emitted); w_proj chunks and out stores ride
the ACT queue (pure loads/stores that never park it at a bad time);
y stores + collectives + constants ride the Pool (SWDGE) queue. A
parked collective never blocks a latency-critical load.

Attention: no max-subtraction softmax (scores ~N(0,1)); row sums via a
per-tile ones-matmul on the PE; reciprocal broadcast applied to y after
the PV matmul. The inner loop is software-pipelined one step (QK_{c+1}
issued before PV_c/R_c). Diagonal 128x512 score tiles restrict
QK/exp/PV/rowsum to the live [128*m:512] column range.

reps>1 repeats the whole kernel body with an all-engine barrier between
reps - used for timing the device portion free of the ~1ms/call axon
tunnel dispatch cost.
"""

from collections import deque

import numpy as np
import ml_dtypes

import concourse.bass as bass
import concourse.mybir as mybir
import concourse.tile as tile
from concourse import bacc, bass_isa
from concourse.bass_utils import run_bass_kernel_spmd

F32 = mybir.dt.float32
BF16 = mybir.dt.bfloat16
NPBF16 = ml_dtypes.bfloat16

B, T, C = 2, 2048, 2048
H, D = 16, 128
NTOK = B * T                     # 4096 flat tokens (batch-major)
SCALE = 1.0 / float(np.sqrt(D))
NCORES = 8
HPC = H // NCORES                # 2 heads per core
FLOC = HPC * D                   # 256 local v features
QK = 512                         # q+k local features (2 heads x 128 x 2)

last_exec_time_ns = None
_cache = {}


def _tri_np():
    # tri[kk, qq] = 1.0 iff kk <= qq  (lower-triangular causal mask, 128x128)
    kk = np.arange(128)[:, None]
    qq = np.arange(128)[None, :]
    return (kk <= qq).astype(NPBF16)


def build_nc(no_collective=False, reps=1):
    nc = bacc.Bacc("TRN2", target_bir_lowering=False, debug=False,
                   num_devices=1 if no_collective else NCORES)

    xt = nc.dram_tensor("xt", [C, NTOK], BF16, kind="ExternalInput")
    wqk = nc.dram_tensor("wqk", [C, QK], BF16, kind="ExternalInput")
    wv = nc.dram_tensor("wv", [C, FLOC], BF16, kind="ExternalInput")
    wp = nc.dram_tensor("wp", [C, C], BF16, kind="ExternalInput")
    out = nc.dram_tensor("out", [512, C], F32, kind="ExternalOutput")

    # per-(batch, head) a2a buffers: 8 shards x [128 feat x 256 tok]
    y_loc = {(b, h): nc.dram_tensor(f"y_loc{b}{h}", [1024, 256], BF16)
             for b in range(B) for h in range(HPC)}
    y_t = {(b, h): nc.dram_tensor(f"y_t{b}{h}", [1024, 256], BF16)
           for b in range(B) for h in range(HPC)}
    tri_dr = nc.inline_tensor(_tri_np(), "tri_c")
    ones_dr = nc.inline_tensor(np.ones((128, 1), NPBF16), "ones_c")
    zeros_dr = nc.inline_tensor(np.zeros((128, 1), np.float32), "zeros_c")

    def a2a(b, h):
        if no_collective:
            nc.gpsimd.dma_start(out=y_t[(b, h)][:, :], in_=y_loc[(b, h)][:, :])
        else:
            nc.gpsimd.collective_compute(
                "AllToAll",
                mybir.AluOpType.bypass,
                replica_groups=[list(range(NCORES))],
                ins=[y_loc[(b, h)][:, :]],
                outs=[y_t[(b, h)][:, :]],
            )

    with tile.TileContext(nc) as tc:
        with tc.tile_pool(name="persist", bufs=1) as persist:
            # q^T,k^T for 2 heads, all tokens: chunk f = {q_h0, q_h1, k_h0, k_h1}
            qk_res = persist.tile([128, 4, NTOK], BF16)
            # v, token-major, per (batch, head): [128 tok, 16 chunks, 128 feat]
            v_pre = {(b, h): persist.tile([128, 16, 128], BF16,
                                          name=f"v_pre{b}{h}")
                     for b in range(B) for h in range(HPC)}
            ones_sb = persist.tile([128, 1], BF16)
            zeros_sb = persist.tile([128, 1], F32)
            tri_sb = persist.tile([128, 128], BF16)
            scr = persist.tile([128, 1], F32)
            # constants ride the Pool (SWDGE) queue; SP queue is reserved for
            # the latency-critical weight/x loads at startup
            nc.gpsimd.dma_start(out=zeros_sb, in_=zeros_dr.ap())
            nc.gpsimd.dma_start(out=ones_sb, in_=ones_dr.ap())
            nc.gpsimd.dma_start(out=tri_sb, in_=tri_dr.ap())
            # warm the ACT exp table set (~2.7us) before attention needs it
            nc.scalar.activation(scr, zeros_sb,
                                 mybir.ActivationFunctionType.Exp,
                                 bias=zeros_sb)

            # long-lived pools open first; phase-1 pools open innermost so
            # they can be released (LIFO) mid-rep to free PSUM banks
            p2p_ctx = tc.tile_pool(name="p2p", bufs=4)
            p2y_ctx = tc.tile_pool(name="p2y", bufs=2)
            p2r_ctx = tc.tile_pool(name="p2r", bufs=2)
            p2ra_ctx = tc.tile_pool(name="p2ra", bufs=2)
            p2pss_ctx = tc.tile_pool(name="p2pss", bufs=4, space="PSUM")
            p2psy_ctx = tc.tile_pool(name="p2psy", bufs=2, space="PSUM")
            p2p = p2p_ctx.__enter__()
            p2y = p2y_ctx.__enter__()
            p2r = p2r_ctx.__enter__()
            p2ra = p2ra_ctx.__enter__()
            p2pss = p2pss_ctx.__enter__()
            p2psy = p2psy_ctx.__enter__()
            p4w_ctx = tc.tile_pool(name="p4w", bufs=8)
            p4y_ctx = tc.tile_pool(name="p4y", bufs=1)
            p4s_ctx = tc.tile_pool(name="p4s", bufs=2)
            p4e_ctx = tc.tile_pool(name="p4e", bufs=16)
            p4w = p4w_ctx.__enter__()
            p4y = p4y_ctx.__enter__()
            p4s = p4s_ctx.__enter__()
            p4e = p4e_ctx.__enter__()

            def attn_steps(b, j):
                """Generator: attention for q-tile j of batch b, both heads.
                Each next() emits one c-iteration (QK+exp+mask+prev-flush);
                the final step per head emits flush/normalize/stores (+a2a
                after the head's last q-tile)."""
                tok0 = b * T
                nk = 4 * j + 4
                for h in range(HPC):
                    v_sb = v_pre[(b, h)]
                    qf, kf = h, 2 + h
                    y_ps = p2psy.tile([128, 512], F32, tag="yps", name="y_ps")
                    # softmax denominators accumulate on the DVE (f32 SBUF)
                    # instead of per-iter ones-matmuls on the PE; the final
                    # partition reduction is one gpsimd all-reduce whose
                    # replicated output feeds reciprocal directly
                    racc = p2ra.tile([128, 512], F32, tag="racc", name="racc")
                    qs = qk_res[:, qf, tok0 + j * 512: tok0 + (j + 1) * 512]
                    pend = None

                    def flush(stop):
                        pc, pp, plo = pend
                        nc.tensor.matmul(
                            y_ps[:, plo:], lhsT=v_sb[:, pc, :],
                            rhs=pp[:, plo:],
                            start=(pc == 0), stop=stop)

                    for c in range(nk):
                        m = c - 4 * j          # >= 0 on diagonal tiles
                        lo = 128 * m if m >= 0 else 0
                        s_ps = p2pss.tile([128, 512], F32, tag="sps",
                                          name="s_ps")
                        nc.tensor.matmul(
                            s_ps[:, lo:],
                            lhsT=qk_res[:, kf,
                                        tok0 + c * 128: tok0 + (c + 1) * 128],
                            rhs=qs[:, lo:],
                            start=True, stop=True,
                        )
                        p_sb = p2p.tile([128, 512], BF16, tag="p", name="p_sb")
                        nc.scalar.activation(
                            p_sb[:, lo:], s_ps[:, lo:],
                            mybir.ActivationFunctionType.Exp,
                            scale=SCALE, bias=zeros_sb,
                        )
                        if m >= 0:
                            # only the leading 128 cols are partial
                            nc.vector.tensor_mul(
                                p_sb[:, lo:lo + 128],
                                p_sb[:, lo:lo + 128], tri_sb)
                        if c == 0:
                            nc.vector.tensor_copy(racc, p_sb)
                        else:
                            nc.vector.tensor_add(
                                racc[:, lo:], racc[:, lo:], p_sb[:, lo:])
                        if pend is not None:
                            flush(stop=False)
                        pend = (c, p_sb, lo)
                        yield
                    flush(stop=True)
                    rb = p2r.tile([128, 512], F32, tag="rb", name="rb")
                    nc.gpsimd.partition_all_reduce(
                        rb, racc, channels=128,
                        reduce_op=bass_isa.ReduceOp.add)
                    nc.vector.reciprocal(rb, rb)
                    y_sb = p2y.tile([128, 512], BF16, tag="ysb", name="y_sb")
                    nc.vector.tensor_mul(y_sb, y_ps, rb)
                    # token eighths 2j, 2j+1 of batch b -> y_loc rows, one
                    # DMA via the d-major view (rows = s*128 + d)
                    nc.gpsimd.dma_start(
                        out=y_loc[(b, h)].ap()
                        .rearrange("(s d) t -> d s t", d=128)[:, 2 * j:2 * j + 2, :],
                        in_=y_sb.rearrange("d (e t) -> d e t", e=2),
                    )
                    if j == 3:
                        a2a(b, h)
                    yield

            # ---- weave machinery: drain attention steps between groups ----
            work = deque()          # generators with steps left

            def drain(n):
                done = 0
                while work and done < n:
                    try:
                        next(work[0])
                        done += 1
                    except StopIteration:
                        work.popleft()
                return done

            def drain_all():
                while work:
                    drain(1 << 20)

            def load_yt(b, eng):
                # MUST be issued after the matching a2a()s (the RAW dep on
                # y_t only exists once the collective is in the program);
                # pick a queue where parking on that dep blocks nothing.
                ybs = []
                for h in range(HPC):
                    yb = p4y.tile([128, 8, 256], BF16, tag=f"yt{b}{h}",
                                  name=f"yt{b}{h}")
                    eng.dma_start(
                        out=yb,
                        in_=y_t[(b, h)].ap().rearrange("(n p) t -> p n t",
                                                       p=128))
                    ybs.append(yb)
                return ybs

            def proj_group(b, ybs, wt, ch, tb, ps_pool):
                ps = ps_pool.tile([128, 256], F32, tag="ops", name="ops")
                for c in range(16):
                    nc.tensor.matmul(
                        ps,
                        lhsT=ybs[c % 2][:, c // 2, tb * 128:(tb + 1) * 128],
                        rhs=wt[:, c, :],
                        start=(c == 0), stop=(c == 15),
                    )
                st = p4s.tile([128, 256], F32, tag="ost", name="ost")
                nc.vector.tensor_copy(st, ps)
                nc.scalar.dma_start(
                    out=out[b * 256 + tb * 128: b * 256 + (tb + 1) * 128,
                            ch * 256:(ch + 1) * 256],
                    in_=st,
                )

            for rep in range(reps):
                if rep:
                    tc.strict_bb_all_engine_barrier()

                p1w_ctx = tc.tile_pool(name="p1w", bufs=1)
                p1x_ctx = tc.tile_pool(name="p1x", bufs=3)
                p1ps_ctx = tc.tile_pool(name="p1ps", bufs=2, space="PSUM")
                p1w = p1w_ctx.__enter__()
                p1x = p1x_ctx.__enter__()
                p1ps = p1ps_ctx.__enter__()
                wqk_sb = p1w.tile([128, 16, QK], BF16)
                wv_sb = p1w.tile([128, 16, FLOC], BF16)

                def p1_tile(bi, ti):
                    """qkv for token tile tt (512 tokens), attention steps of
                    the previous q-tile woven between the 8 matmul groups."""
                    tt = 4 * bi + ti
                    xh = [p1x.tile([128, 8, 512], BF16, tag="xh", name="xbuf")
                          for half in range(2)]
                    if tt == 0 and rep == 0:
                        # first tile: interleave weight/x eighths so the
                        # first matmul group starts ~6us earlier
                        for piece in range(8):
                            nc.sync.dma_start(
                                out=wqk_sb[:, piece * 2:(piece + 1) * 2, :],
                                in_=wqk[piece * 256:(piece + 1) * 256, :]
                                .rearrange("(n p) f -> p n f", p=128))
                            half, sub = piece // 4, piece % 4
                            c0 = half * 8 + sub * 2
                            nc.sync.dma_start(
                                out=xh[half][:, sub * 2:(sub + 1) * 2, :],
                                in_=xt[c0 * 128:(c0 + 2) * 128, 0:512]
                                .rearrange("(n p) f -> p n f", p=128))
                        nc.sync.dma_start(
                            out=wv_sb,
                            in_=wv.ap().rearrange("(n p) f -> p n f", p=128))
                    else:
                        if tt == 0:
                            nc.sync.dma_start(
                                out=wqk_sb,
                                in_=wqk.ap().rearrange("(n p) f -> p n f",
                                                       p=128))
                            nc.sync.dma_start(
                                out=wv_sb,
                                in_=wv.ap().rearrange("(n p) f -> p n f",
                                                      p=128))
                        for half in range(2):
                            nc.sync.dma_start(
                                out=xh[half],
                                in_=xt[half * 1024:(half + 1) * 1024,
                                       tt * 512:(tt + 1) * 512]
                                .rearrange("(n p) f -> p n f", p=128))

                    for fb in range(4):
                        ps = p1ps.tile([128, 512], F32, tag="ps", name="ps")
                        for c in range(16):
                            nc.tensor.matmul(
                                ps,
                                lhsT=wqk_sb[:, c, fb * 128:(fb + 1) * 128],
                                rhs=xh[c // 8][:, c % 8, :],
                                start=(c == 0), stop=(c == 15),
                            )
                        nc.vector.tensor_copy(
                            qk_res[:, fb, tt * 512:(tt + 1) * 512], ps)
                        steps_left[0] -= drain(
                            (steps_left[0] + (7 - fb)) // (8 - fb))
                    for tb in range(4):
                        psv = p1ps.tile([128, 512], F32, tag="ps", name="ps")
                        for c in range(16):
                            nc.tensor.matmul(
                                psv[:, :FLOC],
                                lhsT=xh[c // 8][:, c % 8,
                                                tb * 128:(tb + 1) * 128],
                                rhs=wv_sb[:, c, :],
                                start=(c == 0), stop=(c == 15),
                            )
                        for h in range(HPC):
                            nc.vector.tensor_copy(
                                v_pre[(bi, h)][:, ti * 4 + tb, :],
                                psv[:, h * 128:(h + 1) * 128])
                        g = 4 + tb
                        if g == 7:
                            steps_left[0] -= drain(1 << 20)
                        else:
                            steps_left[0] -= drain(
                                (steps_left[0] + (7 - g)) // (8 - g))

                # ================= schedule =================
                steps_left = [0]
                yt0 = yt1 = None
                for bi in range(B):
                    for ti in range(4):
                        p1_tile(bi, ti)
                        work.append(attn_steps(bi, ti))
                        steps_left[0] += 2 * (4 * ti + 4 + 1)
                        if bi == 1:
                            # w_proj chunks stream on the idle ACT queue
                            # during batch-1 qkv (3+3+2 across t5..t7)
                            if ti >= 1:
                                n0 = [0, 0, 3, 6][ti]
                                n1 = [0, 3, 6, 8][ti]
                                for ch in range(n0, n1):
                                    wt = p4w.tile([128, 16, 256], BF16,
                                                  tag="wp", name=f"wp_t{ch}")
                                    nc.scalar.dma_start(
                                        out=wt,
                                        in_=wp[:, ch * 256:(ch + 1) * 256]
                                        .rearrange("(n p) f -> p n f", p=128))
                                    wp_tiles.append(wt)
                    if bi == 0:
                        wp_tiles = []

                # phase-1 pools done (LIFO); free their PSUM banks. The
                # remaining woven steps (A1 j=3) only touch p2 pools.
                p1ps_ctx.__exit__(None, None, None)
                p1x_ctx.__exit__(None, None, None)
                p1w_ctx.__exit__(None, None, None)
                p4ps_ctx = tc.tile_pool(name="p4ps", bufs=2, space="PSUM")
                p4ps = p4ps_ctx.__enter__()

                # A1 j=3 front-loaded; its tail-drain bubbles filled by the
                # first proj(b0) groups; the a2a(1,*) flights are covered by
                # the rest of proj(b0) plus proj(b1)'s even-c half (which
                # only needs yt1[h0], arriving one a2a earlier)
                # batch-0's a2as (emitted in t4's weave) completed long ago;
                # the SP queue is empty, so these load immediately
                yt0 = load_yt(0, nc.sync)
                while steps_left[0] > 4:
                    steps_left[0] -= drain(1)
                pg = [(ch, tb) for ch in range(8) for tb in range(2)]
                for i, (ch, tb) in enumerate(pg[:4]):
                    proj_group(0, yt0, wp_tiles[ch], ch, tb, p4ps)
                    steps_left[0] -= drain(1)
                drain_all()
                steps_left[0] = 0
                yt1 = load_yt(1, nc.sync)
                for ch, tb in pg[4:]:
                    proj_group(0, yt0, wp_tiles[ch], ch, tb, p4ps)
                # proj(b1) pass 1: even c chunks (head-0 features)
                partials = []
                for ch, tb in pg:
                    ps = p4ps.tile([128, 256], F32, tag="ops", name="ops")
                    for i, c in enumerate(range(0, 16, 2)):
                        nc.tensor.matmul(
                            ps,
                            lhsT=yt1[0][:, c // 2, tb * 128:(tb + 1) * 128],
                            rhs=wp_tiles[ch][:, c, :],
                            start=(i == 0), stop=(i == 7),
                        )
                    pe = p4e.tile([128, 256], BF16, tag="pe", name="pe")
                    nc.vector.tensor_copy(pe, ps)
                    partials.append(pe)
                # pass 2: odd c chunks (head-1 features) + combine
                for (ch, tb), pe in zip(pg, partials):
                    ps = p4ps.tile([128, 256], F32, tag="ops", name="ops")
                    for i, c in enumerate(range(1, 16, 2)):
                        nc.tensor.matmul(
                            ps,
                            lhsT=yt1[1][:, c // 2, tb * 128:(tb + 1) * 128],
                            rhs=wp_tiles[ch][:, c, :],
                            start=(i == 0), stop=(i == 7),
                        )
                    st = p4s.tile([128, 256], F32, tag="ost", name="ost")
                    nc.vector.tensor_add(st, ps, pe)
                    nc.scalar.dma_start(
                        out=out[256 + tb * 128: 256 + (tb + 1) * 128,
                                ch * 256:(ch + 1) * 256],
                        in_=st,
                    )
                p4ps_ctx.__exit__(None, None, None)

            for ctx in (p4e_ctx, p4s_ctx, p4y_ctx, p4w_ctx, p2psy_ctx,
                        p2pss_ctx, p2ra_ctx, p2r_ctx, p2y_ctx, p2p_ctx):
                ctx.__exit__(None, None, None)

    nc.compile()
    return nc


def _in_maps(x, w_attn, w_proj):
    xt = np.ascontiguousarray(
        x.reshape(NTOK, C).T).astype(NPBF16)              # [C, NTOK]
    wp = np.ascontiguousarray(w_proj).astype(NPBF16)
    maps = []
    for i in range(NCORES):
        qcols = w_attn[:, FLOC * i: FLOC * (i + 1)]
        kcols = w_attn[:, C + FLOC * i: C + FLOC * (i + 1)]
        vcols = w_attn[:, 2 * C + FLOC * i: 2 * C + FLOC * (i + 1)]
        maps.append({
            "xt": xt,
            "wqk": np.ascontiguousarray(
                np.concatenate([qcols, kcols], axis=1)).astype(NPBF16),
            "wv": np.ascontiguousarray(vcols).astype(NPBF16),
            "wp": wp,
        })
    return maps


def kernel(x, w_attn, w_proj):
    global last_exec_time_ns
    x = np.asarray(x, dtype=np.float32)
    w_attn = np.asarray(w_attn, dtype=np.float32)
    w_proj = np.asarray(w_proj, dtype=np.float32)

    if "nc" not in _cache:
        _cache["nc"] = build_nc()
    nc = _cache["nc"]

    res = run_bass_kernel_spmd(nc, _in_maps(x, w_attn, w_proj),
                               list(range(NCORES)))
    last_exec_time_ns = res.exec_time_ns

    return assemble([res.results[g]["out"] for g in range(NCORES)])


def assemble(outs):
    # core g's out rows: [0:256] = batch0 tokens [256g:256(g+1)],
    #                    [256:512] = batch1 tokens [256g:256(g+1)]
    full = np.empty((B, T, C), np.float32)
    for g in range(NCORES):
        for b in range(B):
            full[b, 256 * g: 256 * (g + 1), :] = outs[g][b * 256:(b + 1) * 256]
    return full
